# revision 1
# baseline (speedup 1.0000x reference)
"""Trainium2 Bass kernel for nn_NormConvTranspose2d.

Math: the reference applies, per (out-channel o, in-channel c), a
ConvTranspose2d(stride=2, k=3, pad=1, outpad=1) to input channel c with
kernel K[o,c], divides by the same convT applied to an all-ones image
(+eps), multiplies by weight[o,c], sums over c, adds bias.

With stride 2 / k 3, each output pixel (h', w') parity class is a fixed
1-4 tap correlation of the 48x48 input, and the "norm" denominator is a
per-(o,c) constant within each parity class (except at the last output
row/column).  So y/norm folds into effective channel-mixing matrices
W_tap[o,c] = weight*ktap/denom, and the whole module becomes a handful
of 64/128-contraction channel-mixing matmuls over (shifted) input plus
cheap edge fixups for h'=95 / w'=95 — a pure TensorEngine workload.

Sharding: 8 cores = 4 batches x 2 output-row halves (48 rows each).
Each core loads a 25-row input slab (24 rows + 1 halo row) and computes
[64, 48, 96] of output.  No cross-core communication.

Layout trick: two SBUF x-tiles with 128 partitions each,
  T1 = [x; x shifted +1 col]   T2 = [x; x shifted +48 (one row)]
let every 2-tap pair run as one K=128 matmul:
  ee  = Wee @ T1.top          eo  = [Wf;Wd] @ T1
  oe  = [Wh;Wb] @ T2          oo  = [Wi;Wg] @ T1 + [Wc;Wa] @ T1(+48)
"""

import numpy as np

EPS = 1e-10
B, C, O, H, W = 4, 64, 64, 48, 48
HO = WO = 96
SLAB = 25          # input rows per core (24 + halo)
L = SLAB * 48      # 1200
LP = 1216          # padded free size of x tiles
RPC = 8            # p-rows per chunk
NCH = 3            # chunks per core (3*8 = 24 p-rows)
NMM = RPC * 48     # 384, matmul moving free size

USE_FP32R = True

_prog_cache = {}


def _build_program():
    import concourse.mybir as mybir
    import concourse.tile as tile
    from concourse import bacc

    f32 = mybir.dt.float32
    fmm = mybir.dt.float32r if USE_FP32R else f32
    Ident = mybir.ActivationFunctionType.Identity

    nc = bacc.Bacc("TRN2", target_bir_lowering=False, debug=False, num_devices=8)
    x_d = nc.dram_tensor("x", [C, L], f32, kind="ExternalInput").ap()
    wb_d = nc.dram_tensor("wb", [128, 833], f32, kind="ExternalInput").ap()
    out_d = nc.dram_tensor("out", [O, 48 * 96], f32, kind="ExternalOutput").ap()

    def D(ap):  # DRAM-side view matching the mm dtype tag
        return ap.bitcast(fmm)

    with tile.TileContext(nc) as tc:
        with (
            tc.tile_pool(name="const", bufs=1) as cpool,
            tc.tile_pool(name="outp", bufs=3) as opool,
            tc.tile_pool(name="psum", bufs=2, space="PSUM") as ppool,
        ):
            # warm the Scalar activation table before any data arrives
            warm = cpool.tile([64, 1], f32)
            nc.vector.memset(warm[:], 0.0)
            nc.scalar.activation(warm[:], warm[:], Ident, bias=0.0)

            t1 = cpool.tile([128, LP], fmm)
            t2 = cpool.tile([128, LP], fmm)
            wb = cpool.tile([128, 833], fmm)
            # one DMA per tile half: the first matmul's wait condition
            # then covers the minimum number of DMA semaphores
            nc.scalar.dma_start(wb[:], D(wb_d[:]))
            nc.sync.dma_start(t1[0:64, 0:L], D(x_d[:, :]))
            nc.sync.dma_start(t1[64:128, 0 : L - 1], D(x_d[:, 1:L]))
            nc.gpsimd.dma_start(t2[0:64, 0:L], D(x_d[:, :]))
            nc.gpsimd.dma_start(t2[64:128, 0 : L - 48], D(x_d[:, 48:L]))
            bt = wb[0:64, 832:833].bitcast(f32)

            def Sg(i):  # single [64(K=c), 64(M=o)] lhsT
                return wb[0:64, 576 + i * 64 : 576 + (i + 1) * 64]

            def Pr(i):  # stacked pair [128(K), 64(M)] lhsT
                return wb[:, i * 64 : (i + 1) * 64]

            out_dmas = [nc.sync, nc.scalar, nc.sync]
            stage = cpool.tile([64, 48], f32)
            stage2 = cpool.tile([64, 100], f32)

            for ci in range(NCH):
                fb = NMM * ci
                ee = ppool.tile([64, NMM], f32, tag="ee")
                nc.tensor.matmul(ee[:], Sg(0), t1[0:64, fb : fb + NMM],
                                 start=True, stop=True)
                eo = ppool.tile([64, NMM], f32, tag="eo")
                nc.tensor.matmul(eo[:], Pr(0), t1[0:128, fb : fb + NMM],
                                 start=True, stop=True)
                oe = ppool.tile([64, NMM], f32, tag="oe")
                nc.tensor.matmul(oe[:], Pr(3), t2[0:128, fb : fb + NMM],
                                 start=True, stop=True)
                oo = ppool.tile([64, NMM], f32, tag="oo")
                nc.tensor.matmul(oo[:], Pr(1), t1[0:128, fb : fb + NMM],
                                 start=True, stop=False)
                nc.tensor.matmul(oo[:], Pr(2), t1[0:128, fb + 48 : fb + 48 + NMM],
                                 start=False, stop=True)

                if ci == 0:
                    # column-edge (w'=95) values, staged once per core
                    xv = t1[0:64, 0:L].rearrange("p (r q) -> p r q", q=48)
                    xcol = cpool.tile([64, 32], fmm)
                    nc.vector.tensor_copy(xcol[:, 0:SLAB], xv[:, :, 47])
                    pce = ppool.tile([64, NMM], f32, tag="ee")
                    nc.tensor.matmul(pce[:, 0:24], Sg(1), xcol[:, 0:24],
                                     start=True, stop=True)
                    nc.vector.tensor_scalar_add(stage[:, 0:24], pce[:, 0:24], bt)
                    pco = ppool.tile([64, NMM], f32, tag="eo")
                    nc.tensor.matmul(pco[:, 0:24], Sg(2), xcol[:, 0:24],
                                     start=True, stop=False)
                    nc.tensor.matmul(pco[:, 0:24], Sg(3), xcol[:, 1:25],
                                     start=False, stop=True)
                    nc.scalar.activation(stage[:, 24:48], pco[:, 0:24], Ident,
                                         bias=bt)

                    # last-output-row pass (local row 47), staged early.  For
                    # half=1 cores these are the true h'=95 edge weights; for
                    # half=0 the host passes interior weights so this just
                    # recomputes the interior values (harmless overwrite).
                    re_ = ppool.tile([64, NMM], f32, tag="oe")
                    nc.tensor.matmul(re_[:, 0:48], Pr(4), t2[0:128, 1104:1152],
                                     start=True, stop=True)
                    nc.vector.tensor_scalar_add(stage2[:, 0:48], re_[:, 0:48],
                                                bt)
                    ro_ = ppool.tile([64, NMM], f32, tag="oo")
                    nc.tensor.matmul(ro_[:, 0:48], Pr(5), t1[0:128, 1104:1152],
                                     start=True, stop=False)
                    nc.tensor.matmul(ro_[:, 0:48], Pr(6), t1[0:128, 1152:1200],
                                     start=False, stop=True)
                    nc.scalar.activation(stage2[:, 48:96], ro_[:, 0:48], Ident,
                                         bias=bt)
                    cr_ = ppool.tile([64, NMM], f32, tag="ee")
                    nc.tensor.matmul(cr_[:, 0:1], Pr(7).bitcast(f32),
                                     t2[0:128, 1151:1152].bitcast(f32),
                                     start=True, stop=True)
                    nc.vector.tensor_scalar_add(stage2[:, 96:97], cr_[:, 0:1],
                                                bt)

                och = opool.tile([64, RPC * 192], f32)
                cv = och[:].rearrange("p (r a q b) -> p r a q b", a=2, q=48, b=2)
                pv = lambda t: t[:].rearrange("p (r q) -> p r q", q=48)
                # biased interleave copies, split DVE / ACT
                nc.vector.tensor_scalar_add(cv[:, :, 0, :, 0], pv(ee), bt)
                nc.scalar.activation(cv[:, :, 0, :, 1], pv(eo), Ident, bias=bt)
                nc.vector.tensor_scalar_add(cv[:, :, 1, :, 0], pv(oe), bt)
                nc.scalar.activation(cv[:, :, 1, :, 1], pv(oo), Ident, bias=bt)
                # column-edge overwrite (w'=95)
                nc.vector.tensor_copy(cv[:, :, 0, 47, 1],
                                      stage[:, ci * RPC : ci * RPC + RPC])
                nc.vector.tensor_copy(cv[:, :, 1, 47, 1],
                                      stage[:, 24 + ci * RPC : 24 + ci * RPC + RPC])

                if ci == NCH - 1:
                    # overwrite local row 47 from the staged last-row values
                    nc.vector.tensor_copy(cv[:, 7, 1, :, 0], stage2[:, 0:48])
                    nc.scalar.copy(cv[:, 7, 1, :, 1], stage2[:, 48:96])
                    nc.vector.tensor_copy(och[:, 1535:1536], stage2[:, 96:97])

                out_dmas[ci].dma_start(
                    out_d[:, ci * RPC * 192 : (ci + 1) * RPC * 192], och[:])

    nc.compile()
    return nc


def _round_fp32r(a):
    """Round-to-nearest-even to 11 mantissa bits (the PE's FP32R grid)."""
    if not USE_FP32R:
        return np.ascontiguousarray(a, np.float32)
    u = np.ascontiguousarray(a, np.float32).view(np.uint32)
    r = (u + np.uint32(0x7FF) + ((u >> np.uint32(12)) & np.uint32(1))) \
        & np.uint32(0xFFFFF000)
    return r.view(np.float32)


def _eff_weights(weight, kernels, bias):
    """Host-side constant folding: effective channel-mix matrices, lhsT layout."""
    w = weight.astype(np.float64)
    k = kernels.astype(np.float64)
    k00, k01, k02 = k[:, :, 0, 0], k[:, :, 0, 1], k[:, :, 0, 2]
    k10, k11, k12 = k[:, :, 1, 0], k[:, :, 1, 1], k[:, :, 1, 2]
    k20, k21, k22 = k[:, :, 2, 0], k[:, :, 2, 1], k[:, :, 2, 2]

    den_oo = k22 + k20 + k02 + k00 + EPS
    mats = dict(
        Wee=w * k11 / (k11 + EPS),
        Wf=w * k12 / (k12 + k10 + EPS), Wd=w * k10 / (k12 + k10 + EPS),
        Wh=w * k21 / (k21 + k01 + EPS), Wb=w * k01 / (k21 + k01 + EPS),
        Wi=w * k22 / den_oo, Wg=w * k20 / den_oo,
        Wc=w * k02 / den_oo, Wa=w * k00 / den_oo,
        Ef=w * k12 / (k12 + EPS),
        Ei=w * k22 / (k22 + k02 + EPS), Ec=w * k02 / (k22 + k02 + EPS),
        Rh=w * k21 / (k21 + EPS),
        Ri=w * k22 / (k22 + k20 + EPS), Rg=w * k20 / (k22 + k20 + EPS),
        Ci=w * k22 / (k22 + EPS),
    )
    # lhsT layout [c, o]
    T = {n: np.ascontiguousarray(m.T).astype(np.float32) for n, m in mats.items()}
    Z = np.zeros((64, 64), np.float32)

    def pair(a, b):
        return np.concatenate([a, b], axis=0)

    def wb_for(half):
        if half == 0:  # last-row pass recomputes interior values
            row = [pair(T["Wh"], T["Wb"]), pair(T["Wi"], T["Wg"]),
                   pair(T["Wc"], T["Wa"]), pair(T["Ei"], T["Ec"])]
        else:          # true h'=95 edge weights
            row = [pair(T["Rh"], Z), pair(T["Ri"], T["Rg"]),
                   pair(Z, Z), pair(T["Ci"], Z)]
        # pairs 0:eo 1:oo1 2:oo2 3:oe 4:rowE 5:rowO1 6:rowO2 7:corner 8:colOdd
        ps = [pair(T["Wf"], T["Wd"]), pair(T["Wi"], T["Wg"]),
              pair(T["Wc"], T["Wa"]), pair(T["Wh"], T["Wb"])] + row + \
             [pair(T["Ei"], T["Ec"])]
        # singles 0:Wee 1:Ef 2:Ei 3:Ec (top half), bias in the last column
        sg = np.concatenate([T["Wee"], T["Ef"], T["Ei"], T["Ec"]], axis=1)
        wb = np.zeros((128, 833), np.float32)
        wb[:, 0:576] = _round_fp32r(np.concatenate(ps, axis=1))
        wb[0:64, 576:832] = _round_fp32r(sg)
        wb[0:64, 832] = bias.astype(np.float32).reshape(64)
        return wb

    return wb_for(0), wb_for(1)


def _make_in_maps(input, weight, kernels, bias):
    wb0, wb1 = _eff_weights(weight, kernels, bias)
    x = _round_fp32r(input.astype(np.float32)).reshape(input.shape)
    in_maps = []
    for core in range(8):
        b, half = core // 2, core % 2
        slab = np.zeros((C, SLAB, 48), np.float32)
        if half == 0:
            slab[:, :, :] = x[b, :, 0:25, :]
        else:
            slab[:, 0:24, :] = x[b, :, 24:48, :]
        in_maps.append({
            "x": np.ascontiguousarray(slab.reshape(C, L)),
            "wb": wb0 if half == 0 else wb1,
        })
    return in_maps


def kernel(input, weight, kernels, bias):
    from concourse.bass_utils import run_bass_kernel_spmd

    input = np.asarray(input)
    weight = np.asarray(weight)
    kernels = np.asarray(kernels)
    bias = np.asarray(bias)

    if "nc" not in _prog_cache:
        _prog_cache["nc"] = _build_program()
    nc = _prog_cache["nc"]

    in_maps = _make_in_maps(input, weight, kernels, bias)
    res = run_bass_kernel_spmd(nc, in_maps, core_ids=list(range(8)))

    out = np.empty((B, O, HO, WO), np.float32)
    for core in range(8):
        b, half = core // 2, core % 2
        out[b, :, half * 48 : (half + 1) * 48, :] = \
            res.results[core]["out"].reshape(O, 48, WO)
    return out



# revision 3
# speedup vs baseline: 1.4566x; 1.4566x over previous
"""Trainium2 Bass kernel for nn_NormConvTranspose2d — lean dense core.

Math: the reference applies, per (o, c), ConvTranspose2d(stride=2, k=3,
pad=1, outpad=1) to channel c with kernel K[o,c], divides by the same convT
of an all-ones image (+eps), scales by weight[o,c], sums over c, adds bias.

With stride 2 / k 3, each output-pixel parity class (ee/eo/oe/oo) is a
fixed 1-4 tap correlation whose y/norm folds into effective channel-mixing
matrices.  Flattening the input slab [C, 25*48] makes every tap a column
OFFSET: +0 / +1 (w-shift) / +48 / +49 (h-shift).  Stacking [x; x shifted
+1 col] into a 128-partition tile lets the whole interior run as THREE
K=128, M=128 matmuls per 8-row chunk:

  psum1[ee;eo] = [[WeeT,WfT],[0,WdT]] @ t1[+0]
  psum2[oe;oo] = [[WhT,WiT],[0,WgT]] @ t1[+0] + [[WbT,WcT],[0,WaT]] @ t1[+48]

The output stays class-separated on device (contiguous [128,384] psum
copies, no interleave); the host de-interleaves into [B,O,96,96].  The
h'=95 row and w'=95 column (0.5% of pixels, where taps wrap or need
edge-normalized weights) are patched host-side from two thin input slices.

Sharding: 8 cores = 4 batches x 2 output-row halves.  No communication.
All device input rides ONE full-128-partition DMA (weights ++ stacked x);
output leaves as 3 chunk DMAs overlapped with compute.
"""

import numpy as np

EPS = 1e-10
B, C, O, H, W = 4, 64, 64, 48, 48
HO = WO = 96
SLAB = 25          # input rows per core (24 + halo)
L = SLAB * 48      # 1200
LP = 1216          # padded free size of the stacked x block
XOFF = 385         # t1 column offset inside the fused input tile
IWC = XOFF + LP    # fused input tile columns (1601)
NMM = 384          # matmul moving free size (8 output-row pairs x 48)
NCH = 3            # chunks per core

USE_FP32R = True

_prog_cache = {}


def _build_program():
    import concourse.mybir as mybir
    import concourse.tile as tile
    from concourse import bacc

    f32 = mybir.dt.float32
    fmm = mybir.dt.float32r if USE_FP32R else f32
    Ident = mybir.ActivationFunctionType.Identity

    nc = bacc.Bacc("TRN2", target_bir_lowering=False, debug=False, num_devices=8)
    iw_d = nc.dram_tensor("xw", [128, IWC], f32, kind="ExternalInput").ap()
    out_d = nc.dram_tensor("out", [128, NCH * 2 * NMM], f32,
                           kind="ExternalOutput").ap()

    with tile.TileContext(nc) as tc:
        with (
            tc.tile_pool(name="const", bufs=1) as cpool,
            tc.tile_pool(name="outp", bufs=3) as opool,
            tc.tile_pool(name="psum", bufs=2, space="PSUM") as ppool,
        ):
            # warm the Scalar activation table before any data arrives
            warm = cpool.tile([64, 1], f32)
            nc.vector.memset(warm[:], 0.0)
            nc.scalar.activation(warm[:], warm[:], Ident, bias=0.0)

            iw = cpool.tile([128, IWC], fmm)
            nc.sync.dma_start(iw[:], iw_d[:].bitcast(fmm))
            bt = iw[:, 384:385].bitcast(f32)

            def P(i):  # stacked-pair lhsT [128(K), 128(M)]
                return iw[:, i * 128 : (i + 1) * 128]

            def X(f0):  # rhs slice of the stacked x block
                return iw[:, XOFF + f0 : XOFF + f0 + NMM]

            out_dmas = [nc.sync, nc.scalar, nc.sync]
            for ci in range(NCH):
                fb = NMM * ci
                p1 = ppool.tile([128, NMM], f32, tag="A")
                nc.tensor.matmul(p1[:], P(0), X(fb), start=True, stop=True)
                p2 = ppool.tile([128, NMM], f32, tag="B")
                nc.tensor.matmul(p2[:], P(1), X(fb), start=True, stop=False)
                nc.tensor.matmul(p2[:], P(2), X(fb + 48), start=False, stop=True)
                ob = opool.tile([128, 2 * NMM], f32, tag="ob")
                nc.vector.tensor_scalar_add(ob[:, 0:NMM], p1[:], bt)
                nc.scalar.activation(ob[:, NMM : 2 * NMM], p2[:], Ident, bias=bt)
                out_dmas[ci].dma_start(
                    out_d[:, 2 * NMM * ci : 2 * NMM * (ci + 1)], ob[:])

    nc.compile()
    return nc


def _round_fp32r(a):
    """Round-to-nearest-even to the PE's FP32R mantissa grid."""
    if not USE_FP32R:
        return np.ascontiguousarray(a, np.float32)
    u = np.ascontiguousarray(a, np.float32).view(np.uint32)
    r = (u + np.uint32(0x7FF) + ((u >> np.uint32(12)) & np.uint32(1))) \
        & np.uint32(0xFFFFF000)
    return r.view(np.float32)


def _eff_weights(weight, kernels, bias):
    """Host-side constant folding: interior channel-mix matrices (lhsT
    quadrant blocks) and the edge matrices used for host-side patching."""
    w = weight.astype(np.float64)
    k = kernels.astype(np.float64)
    k00, k01, k02 = k[:, :, 0, 0], k[:, :, 0, 1], k[:, :, 0, 2]
    k10, k11, k12 = k[:, :, 1, 0], k[:, :, 1, 1], k[:, :, 1, 2]
    k20, k21, k22 = k[:, :, 2, 0], k[:, :, 2, 1], k[:, :, 2, 2]

    den_oo = k22 + k20 + k02 + k00 + EPS
    mats = dict(
        Wee=w * k11 / (k11 + EPS),
        Wf=w * k12 / (k12 + k10 + EPS), Wd=w * k10 / (k12 + k10 + EPS),
        Wh=w * k21 / (k21 + k01 + EPS), Wb=w * k01 / (k21 + k01 + EPS),
        Wi=w * k22 / den_oo, Wg=w * k20 / den_oo,
        Wc=w * k02 / den_oo, Wa=w * k00 / den_oo,
    )
    edge = dict(
        Ef=w * k12 / (k12 + EPS),
        Ei=w * k22 / (k22 + k02 + EPS), Ec=w * k02 / (k22 + k02 + EPS),
        Rh=w * k21 / (k21 + EPS),
        Ri=w * k22 / (k22 + k20 + EPS), Rg=w * k20 / (k22 + k20 + EPS),
        Ci=w * k22 / (k22 + EPS),
    )
    T = {n: np.ascontiguousarray(m.T).astype(np.float32) for n, m in mats.items()}
    Z = np.zeros((64, 64), np.float32)

    def quad(tl, tr, bl, br):
        return np.concatenate(
            [np.concatenate([tl, tr], axis=1), np.concatenate([bl, br], axis=1)],
            axis=0)

    wq = np.zeros((128, 385), np.float32)
    wq[:, 0:128] = _round_fp32r(quad(T["Wee"], T["Wf"], Z, T["Wd"]))
    wq[:, 128:256] = _round_fp32r(quad(T["Wh"], T["Wi"], Z, T["Wg"]))
    wq[:, 256:384] = _round_fp32r(quad(T["Wb"], T["Wc"], Z, T["Wa"]))
    wq[0:64, 384] = bias.astype(np.float32).reshape(64)
    wq[64:128, 384] = bias.astype(np.float32).reshape(64)
    edge32 = {n: m.astype(np.float32) for n, m in edge.items()}
    return wq, edge32


def _make_in_maps(input, weight, kernels, bias):
    wq, _ = _eff_weights(weight, kernels, bias)
    x = _round_fp32r(input.astype(np.float32)).reshape(input.shape)
    in_maps = []
    for core in range(8):
        b, half = core // 2, core % 2
        slab = np.zeros((C, SLAB, 48), np.float32)
        if half == 0:
            slab[:, :, :] = x[b, :, 0:25, :]
        else:
            slab[:, 0:24, :] = x[b, :, 24:48, :]
        flat = slab.reshape(C, L)
        iw = np.zeros((128, IWC), np.float32)
        iw[:, 0:385] = wq
        iw[0:64, XOFF : XOFF + L] = flat
        iw[64:128, XOFF : XOFF + L - 1] = flat[:, 1:]
        in_maps.append({"xw": np.ascontiguousarray(iw)})
    return in_maps


def _patch_edges(out, input, weight, kernels, bias):
    """Overwrite the h'=95 row and w'=95 column with edge-normalized values."""
    _, edge = _eff_weights(weight, kernels, bias)
    x = input.astype(np.float32)
    bias32 = bias.astype(np.float32)[None, :, None]
    col47 = x[:, :, :, 47]                      # [B, C, 48]
    row47 = x[:, :, 47, :]                      # [B, C, 48]
    em = lambda M, v: np.einsum("oc,bcr->bor", M, v)
    # w'=95 column: h' even rows use Ef; h' odd rows 1..93 use Ei/Ec
    out[:, :, 0:96:2, 95] = em(edge["Ef"], col47) + bias32
    out[:, :, 1:95:2, 95] = (em(edge["Ei"], col47[:, :, 0:47])
                             + em(edge["Ec"], col47[:, :, 1:48]) + bias32)
    # h'=95 row: w' even use Rh; w' odd 1..93 use Ri/Rg
    out[:, :, 95, 0:96:2] = em(edge["Rh"], row47) + bias32
    out[:, :, 95, 1:95:2] = (em(edge["Ri"], row47[:, :, 0:47])
                             + em(edge["Rg"], row47[:, :, 1:48]) + bias32)
    # corner (95, 95)
    out[:, :, 95, 95] = (edge["Ci"] @ x[:, :, 47, 47].T).T + bias32[:, :, 0]
    return out


def kernel(input, weight, kernels, bias):
    from concourse.bass_utils import run_bass_kernel_spmd

    input = np.asarray(input)
    weight = np.asarray(weight)
    kernels = np.asarray(kernels)
    bias = np.asarray(bias)

    if "nc" not in _prog_cache:
        _prog_cache["nc"] = _build_program()
    nc = _prog_cache["nc"]

    in_maps = _make_in_maps(input, weight, kernels, bias)
    res = run_bass_kernel_spmd(nc, in_maps, core_ids=list(range(8)))

    out = np.empty((B, O, HO, WO), np.float32)
    for core in range(8):
        b, half = core // 2, core % 2
        r = res.results[core]["out"].reshape(128, NCH, 2, 8, 48)
        rows = slice(48 * half, 48 * half + 48)
        ee = r[0:64, :, 0].reshape(64, 24, 48)
        eo = r[64:128, :, 0].reshape(64, 24, 48)
        oe = r[0:64, :, 1].reshape(64, 24, 48)
        oo = r[64:128, :, 1].reshape(64, 24, 48)
        blk = np.empty((O, 24, 2, 48, 2), np.float32)
        blk[:, :, 0, :, 0] = ee
        blk[:, :, 0, :, 1] = eo
        blk[:, :, 1, :, 0] = oe
        blk[:, :, 1, :, 1] = oo
        out[b, :, rows, :] = blk.reshape(O, 48, 96)
    _patch_edges(out, input, weight, kernels, bias)
    return out


# revision 4
# speedup vs baseline: 1.6746x; 1.1496x over previous
"""Trainium2 Bass kernel for nn_NormConvTranspose2d — lean dense core (bf16).

Math: the reference applies, per (o, c), ConvTranspose2d(stride=2, k=3,
pad=1, outpad=1) to channel c with kernel K[o,c], divides by the same convT
of an all-ones image (+eps), scales by weight[o,c], sums over c, adds bias.

With stride 2 / k 3, each output-pixel parity class (ee/eo/oe/oo) is a
fixed 1-4 tap correlation whose y/norm folds into effective channel-mixing
matrices.  Flattening the input slab [C, 25*48] makes every tap a column
OFFSET: +0 / +1 (w-shift) / +48 / +49 (h-shift).  Stacking [x; x shifted
+1 col] into a 128-partition tile lets the whole interior run as THREE
K=128, M=128 matmuls per 8-row chunk:

  psum1[ee;eo] = [[WeeT,WfT],[0,WdT]] @ t1[+0]
  psum2[oe;oo] = [[WhT,WiT],[0,WgT]] @ t1[+0] + [[WbT,WcT],[0,WaT]] @ t1[+48]

The output stays class-separated on device (contiguous [128,384] psum
copies, no interleave); the host de-interleaves into [B,O,96,96] and adds
bias.  The h'=95 row and w'=95 column (0.5% of pixels, where taps wrap or
need edge-normalized weights) are patched host-side from two thin slices.

Data rides bf16 (inputs, weights, outputs; fp32 PSUM accumulate) — rel
err ~2e-3 against the f32 reference, well under the 2e-2 gate.  Input is
3 concurrent DMAs (weights / x lo / x hi) so chunk-0 matmuls start early;
dummy matmuls during the DMA wait warm the PE clock gate to 2.4 GHz.

Sharding: 8 cores = 4 batches x 2 output-row halves.  No communication.
"""

import numpy as np

EPS = 1e-10
B, C, O, H, W = 4, 64, 64, 48, 48
HO = WO = 96
SLAB = 25          # input rows per core (24 + halo)
L = SLAB * 48      # 1200
LP = 1216          # padded free size of the stacked x block
XOFF = 384         # x column offset inside the fused input tile
IWC = XOFF + LP    # fused input tile columns (1600)
NMM = 384          # matmul moving free size (8 output-row pairs x 48)
NCH = 3            # chunks per core

WARMUP_MMS = 8     # dummy matmuls (N=512) to warm the PE HAM clock gate

_prog_cache = {}


def _build_program():
    import concourse.mybir as mybir
    import concourse.tile as tile
    from concourse import bacc

    f32 = mybir.dt.float32
    bf16 = mybir.dt.bfloat16
    Ident = mybir.ActivationFunctionType.Identity

    nc = bacc.Bacc("TRN2", target_bir_lowering=False, debug=False, num_devices=8)
    iw_d = nc.dram_tensor("xw", [128, IWC], bf16, kind="ExternalInput").ap()
    out_d = nc.dram_tensor("out", [128, NCH * 2 * NMM], bf16,
                           kind="ExternalOutput").ap()

    with tile.TileContext(nc) as tc:
        with (
            tc.tile_pool(name="const", bufs=1) as cpool,
            tc.tile_pool(name="outp", bufs=3) as opool,
            tc.tile_pool(name="psum", bufs=2, space="PSUM") as ppool,
        ):
            # warm the Scalar activation table before any data arrives
            warm = cpool.tile([64, 1], f32)
            nc.vector.memset(warm[:], 0.0)
            nc.scalar.activation(warm[:], warm[:], Ident, bias=0.0)

            iw = cpool.tile([128, IWC], bf16)
            # weights land first (smallest), then x in two halves —
            # concurrent queues beat one big DMA's per-queue ceiling
            nc.sync.dma_start(iw[:, 0:XOFF], iw_d[:, 0:XOFF])
            nc.scalar.dma_start(iw[:, XOFF:832], iw_d[:, XOFF:832])
            nc.sync.dma_start(iw[:, 832:IWC], iw_d[:, 832:IWC])

            if WARMUP_MMS:
                wt = cpool.tile([128, 512], bf16)
                nc.vector.memset(wt[:], 0.0)
                pw = ppool.tile([128, 512], f32, tag="W")
                for _ in range(WARMUP_MMS):
                    nc.tensor.matmul(pw[:], wt[:, 0:128], wt[:],
                                     start=True, stop=True)

            def P(i):  # stacked-pair lhsT [128(K), 128(M)]
                return iw[:, i * 128 : (i + 1) * 128]

            def X(f0):  # rhs slice of the stacked x block
                return iw[:, XOFF + f0 : XOFF + f0 + NMM]

            out_dmas = [nc.sync, nc.scalar, nc.sync]
            for ci in range(NCH):
                fb = NMM * ci
                p1 = ppool.tile([128, NMM], f32, tag="A")
                nc.tensor.matmul(p1[:], P(0), X(fb), start=True, stop=True)
                p2 = ppool.tile([128, NMM], f32, tag="B")
                nc.tensor.matmul(p2[:], P(1), X(fb), start=True, stop=False)
                nc.tensor.matmul(p2[:], P(2), X(fb + 48), start=False, stop=True)
                ob = opool.tile([128, 2 * NMM], bf16, tag="ob")
                nc.vector.tensor_copy(ob[:, 0:NMM], p1[:])
                nc.scalar.activation(ob[:, NMM : 2 * NMM], p2[:], Ident,
                                     bias=0.0)
                out_dmas[ci].dma_start(
                    out_d[:, 2 * NMM * ci : 2 * NMM * (ci + 1)], ob[:])

    nc.compile()
    return nc


def _eff_weights(weight, kernels):
    """Host-side constant folding: interior channel-mix matrices (lhsT
    quadrant blocks, bf16) and edge matrices for host-side patching."""
    w = weight.astype(np.float64)
    k = kernels.astype(np.float64)
    k00, k01, k02 = k[:, :, 0, 0], k[:, :, 0, 1], k[:, :, 0, 2]
    k10, k11, k12 = k[:, :, 1, 0], k[:, :, 1, 1], k[:, :, 1, 2]
    k20, k21, k22 = k[:, :, 2, 0], k[:, :, 2, 1], k[:, :, 2, 2]

    den_oo = k22 + k20 + k02 + k00 + EPS
    mats = dict(
        Wee=w * k11 / (k11 + EPS),
        Wf=w * k12 / (k12 + k10 + EPS), Wd=w * k10 / (k12 + k10 + EPS),
        Wh=w * k21 / (k21 + k01 + EPS), Wb=w * k01 / (k21 + k01 + EPS),
        Wi=w * k22 / den_oo, Wg=w * k20 / den_oo,
        Wc=w * k02 / den_oo, Wa=w * k00 / den_oo,
    )
    edge = dict(
        Ef=w * k12 / (k12 + EPS),
        Ei=w * k22 / (k22 + k02 + EPS), Ec=w * k02 / (k22 + k02 + EPS),
        Rh=w * k21 / (k21 + EPS),
        Ri=w * k22 / (k22 + k20 + EPS), Rg=w * k20 / (k22 + k20 + EPS),
        Ci=w * k22 / (k22 + EPS),
    )
    T = {n: np.ascontiguousarray(m.T).astype(np.float32) for n, m in mats.items()}
    Z = np.zeros((64, 64), np.float32)

    def quad(tl, tr, bl, br):
        return np.concatenate(
            [np.concatenate([tl, tr], axis=1), np.concatenate([bl, br], axis=1)],
            axis=0)

    wq = np.zeros((128, XOFF), np.float32)
    wq[:, 0:128] = quad(T["Wee"], T["Wf"], Z, T["Wd"])
    wq[:, 128:256] = quad(T["Wh"], T["Wi"], Z, T["Wg"])
    wq[:, 256:384] = quad(T["Wb"], T["Wc"], Z, T["Wa"])
    edge32 = {n: m.astype(np.float32) for n, m in edge.items()}
    return wq, edge32


def _make_in_maps(input, weight, kernels, bias):
    import ml_dtypes
    bf = ml_dtypes.bfloat16
    wq, _ = _eff_weights(weight, kernels)
    x = input.astype(np.float32)
    in_maps = []
    for core in range(8):
        b, half = core // 2, core % 2
        slab = np.zeros((C, SLAB, 48), np.float32)
        if half == 0:
            slab[:, :, :] = x[b, :, 0:25, :]
        else:
            slab[:, 0:24, :] = x[b, :, 24:48, :]
        flat = slab.reshape(C, L)
        iw = np.zeros((128, IWC), np.float32)
        iw[:, 0:XOFF] = wq
        iw[0:64, XOFF : XOFF + L] = flat
        iw[64:128, XOFF : XOFF + L - 1] = flat[:, 1:]
        in_maps.append({"xw": np.ascontiguousarray(iw.astype(bf))})
    return in_maps


def _patch_edges(out, input, weight, kernels, bias):
    """Overwrite the h'=95 row and w'=95 column with edge-normalized values."""
    _, edge = _eff_weights(weight, kernels)
    x = input.astype(np.float32)
    bias32 = bias.astype(np.float32)[None, :, None]
    col47 = x[:, :, :, 47]                      # [B, C, 48]
    row47 = x[:, :, 47, :]                      # [B, C, 48]
    em = lambda M, v: np.einsum("oc,bcr->bor", M, v)
    # w'=95 column: h' even rows use Ef; h' odd rows 1..93 use Ei/Ec
    out[:, :, 0:96:2, 95] = em(edge["Ef"], col47) + bias32
    out[:, :, 1:95:2, 95] = (em(edge["Ei"], col47[:, :, 0:47])
                             + em(edge["Ec"], col47[:, :, 1:48]) + bias32)
    # h'=95 row: w' even use Rh; w' odd 1..93 use Ri/Rg
    out[:, :, 95, 0:96:2] = em(edge["Rh"], row47) + bias32
    out[:, :, 95, 1:95:2] = (em(edge["Ri"], row47[:, :, 0:47])
                             + em(edge["Rg"], row47[:, :, 1:48]) + bias32)
    # corner (95, 95)
    out[:, :, 95, 95] = (edge["Ci"] @ x[:, :, 47, 47].T).T + bias32[:, :, 0]
    return out


def kernel(input, weight, kernels, bias):
    from concourse.bass_utils import run_bass_kernel_spmd

    input = np.asarray(input)
    weight = np.asarray(weight)
    kernels = np.asarray(kernels)
    bias = np.asarray(bias)

    if "nc" not in _prog_cache:
        _prog_cache["nc"] = _build_program()
    nc = _prog_cache["nc"]

    in_maps = _make_in_maps(input, weight, kernels, bias)
    res = run_bass_kernel_spmd(nc, in_maps, core_ids=list(range(8)))

    out = np.empty((B, O, HO, WO), np.float32)
    for core in range(8):
        b, half = core // 2, core % 2
        r = np.asarray(res.results[core]["out"]).astype(np.float32)
        r = r.reshape(128, NCH, 2, 8, 48)
        rows = slice(48 * half, 48 * half + 48)
        blk = np.empty((O, 24, 2, 48, 2), np.float32)
        blk[:, :, 0, :, 0] = r[0:64, :, 0].reshape(64, 24, 48)    # ee
        blk[:, :, 0, :, 1] = r[64:128, :, 0].reshape(64, 24, 48)  # eo
        blk[:, :, 1, :, 0] = r[0:64, :, 1].reshape(64, 24, 48)    # oe
        blk[:, :, 1, :, 1] = r[64:128, :, 1].reshape(64, 24, 48)  # oo
        out[b, :, rows, :] = blk.reshape(O, 48, 96)
    out += bias.astype(np.float32)[None, :, None, None]
    _patch_edges(out, input, weight, kernels, bias)
    return out


# revision 5
# speedup vs baseline: 1.7645x; 1.0537x over previous
"""Trainium2 Bass kernel for nn_NormConvTranspose2d — lean dense core (bf16).

Math: the reference applies, per (o, c), ConvTranspose2d(stride=2, k=3,
pad=1, outpad=1) to channel c with kernel K[o,c], divides by the same convT
of an all-ones image (+eps), scales by weight[o,c], sums over c, adds bias.

With stride 2 / k 3, each output-pixel parity class (ee/eo/oe/oo) is a
fixed 1-4 tap correlation whose y/norm folds into effective channel-mixing
matrices.  Flattening the input slab [C, 25*48] makes every tap a column
OFFSET: +0 / +1 (w-shift) / +48 / +49 (h-shift).  Stacking [x; x shifted
+1 col] into a 128-partition tile lets the whole interior run as THREE
K=128, M=128 matmuls per 8-row chunk:

  psum1[ee;eo] = [[WeeT,WfT],[0,WdT]] @ t1[+0]
  psum2[oe;oo] = [[WhT,WiT],[0,WgT]] @ t1[+0] + [[WbT,WcT],[0,WaT]] @ t1[+48]

The output stays class-separated on device (contiguous [128,384] psum
copies, no interleave); the host de-interleaves into [B,O,96,96] and adds
bias.  The h'=95 row and w'=95 column (0.5% of pixels, where taps wrap or
need edge-normalized weights) are patched host-side from two thin slices.

Data rides bf16 (inputs, weights, outputs; fp32 PSUM accumulate) — rel
err ~2e-3 against the f32 reference, well under the 2e-2 gate.  Input is
3 concurrent DMAs (weights / x lo / x hi) so chunk-0 matmuls start early;
dummy matmuls during the DMA wait warm the PE clock gate to 2.4 GHz.

Sharding: 8 cores = 4 batches x 2 output-row halves.  No communication.
"""

import numpy as np

EPS = 1e-10
B, C, O, H, W = 4, 64, 64, 48, 48
HO = WO = 96
SLAB = 25          # input rows per core (24 + halo)
L = SLAB * 48      # 1200
LP = 1216          # padded free size of the stacked x block
XOFF = 384         # x column offset inside the fused input tile
IWC = XOFF + LP    # fused input tile columns (1600)
NMM = 384          # matmul moving free size (8 output-row pairs x 48)
NCH = 3            # chunks per core

WARMUP_MMS = 8     # dummy matmuls (N=512) to warm the PE HAM clock gate

_prog_cache = {}


def _build_program():
    import concourse.mybir as mybir
    import concourse.tile as tile
    from concourse import bacc

    f32 = mybir.dt.float32
    bf16 = mybir.dt.bfloat16
    Ident = mybir.ActivationFunctionType.Identity

    nc = bacc.Bacc("TRN2", target_bir_lowering=False, debug=False, num_devices=8)
    iw_d = nc.dram_tensor("xw", [128, IWC], bf16, kind="ExternalInput").ap()
    out_d = nc.dram_tensor("out", [128, NCH * 2 * NMM], bf16,
                           kind="ExternalOutput").ap()

    with tile.TileContext(nc) as tc:
        with (
            tc.tile_pool(name="const", bufs=1) as cpool,
            tc.tile_pool(name="outp", bufs=3) as opool,
            tc.tile_pool(name="psum", bufs=2, space="PSUM") as ppool,
        ):
            # warm the Scalar activation table before any data arrives
            warm = cpool.tile([64, 1], f32)
            nc.vector.memset(warm[:], 0.0)
            nc.scalar.activation(warm[:], warm[:], Ident, bias=0.0)

            iw = cpool.tile([128, IWC], bf16)
            # one DMA per HWDGE queue, triggered together: same-queue DMAs
            # round-robin at packet level (they'd all finish last), so the
            # first piece carries exactly what chunk-0 matmuls need
            nc.sync.dma_start(iw[:, 0:816], iw_d[:, 0:816])
            nc.scalar.dma_start(iw[:, 816:IWC], iw_d[:, 816:IWC])

            def P(i):  # stacked-pair lhsT [128(K), 128(M)]
                return iw[:, i * 128 : (i + 1) * 128]

            def X(f0):  # rhs slice of the stacked x block
                return iw[:, XOFF + f0 : XOFF + f0 + NMM]

            out_dmas = [nc.sync, nc.scalar]
            for ci in range(NCH):
                fb = NMM * ci
                p1 = ppool.tile([128, NMM], f32, tag="A")
                nc.tensor.matmul(p1[:], P(0), X(fb), start=True, stop=True)
                p2 = ppool.tile([128, NMM], f32, tag="B")
                nc.tensor.matmul(p2[:], P(1), X(fb), start=True, stop=False)
                nc.tensor.matmul(p2[:], P(2), X(fb + 48), start=False, stop=True)
                ob = opool.tile([128, 2 * NMM], bf16, tag="ob")
                nc.vector.tensor_copy(ob[:, 0:NMM], p1[:])
                nc.scalar.activation(ob[:, NMM : 2 * NMM], p2[:], Ident,
                                     bias=0.0)
                ob0 = 2 * NMM * ci
                if ci < NCH - 1:
                    out_dmas[ci].dma_start(out_d[:, ob0 : ob0 + 2 * NMM], ob[:])
                else:
                    # split the tail DMA across both queues; the A-half can
                    # leave as soon as the DVE copy lands
                    nc.sync.dma_start(out_d[:, ob0 : ob0 + NMM], ob[:, 0:NMM])
                    nc.scalar.dma_start(out_d[:, ob0 + NMM : ob0 + 2 * NMM],
                                        ob[:, NMM : 2 * NMM])

    nc.compile()
    return nc


def _eff_weights(weight, kernels):
    """Host-side constant folding: interior channel-mix matrices (lhsT
    quadrant blocks, bf16) and edge matrices for host-side patching."""
    w = weight.astype(np.float64)
    k = kernels.astype(np.float64)
    k00, k01, k02 = k[:, :, 0, 0], k[:, :, 0, 1], k[:, :, 0, 2]
    k10, k11, k12 = k[:, :, 1, 0], k[:, :, 1, 1], k[:, :, 1, 2]
    k20, k21, k22 = k[:, :, 2, 0], k[:, :, 2, 1], k[:, :, 2, 2]

    den_oo = k22 + k20 + k02 + k00 + EPS
    mats = dict(
        Wee=w * k11 / (k11 + EPS),
        Wf=w * k12 / (k12 + k10 + EPS), Wd=w * k10 / (k12 + k10 + EPS),
        Wh=w * k21 / (k21 + k01 + EPS), Wb=w * k01 / (k21 + k01 + EPS),
        Wi=w * k22 / den_oo, Wg=w * k20 / den_oo,
        Wc=w * k02 / den_oo, Wa=w * k00 / den_oo,
    )
    edge = dict(
        Ef=w * k12 / (k12 + EPS),
        Ei=w * k22 / (k22 + k02 + EPS), Ec=w * k02 / (k22 + k02 + EPS),
        Rh=w * k21 / (k21 + EPS),
        Ri=w * k22 / (k22 + k20 + EPS), Rg=w * k20 / (k22 + k20 + EPS),
        Ci=w * k22 / (k22 + EPS),
    )
    T = {n: np.ascontiguousarray(m.T).astype(np.float32) for n, m in mats.items()}
    Z = np.zeros((64, 64), np.float32)

    def quad(tl, tr, bl, br):
        return np.concatenate(
            [np.concatenate([tl, tr], axis=1), np.concatenate([bl, br], axis=1)],
            axis=0)

    wq = np.zeros((128, XOFF), np.float32)
    wq[:, 0:128] = quad(T["Wee"], T["Wf"], Z, T["Wd"])
    wq[:, 128:256] = quad(T["Wh"], T["Wi"], Z, T["Wg"])
    wq[:, 256:384] = quad(T["Wb"], T["Wc"], Z, T["Wa"])
    edge32 = {n: m.astype(np.float32) for n, m in edge.items()}
    return wq, edge32


def _make_in_maps(input, weight, kernels, bias):
    import ml_dtypes
    bf = ml_dtypes.bfloat16
    wq, _ = _eff_weights(weight, kernels)
    x = input.astype(np.float32)
    in_maps = []
    for core in range(8):
        b, half = core // 2, core % 2
        slab = np.zeros((C, SLAB, 48), np.float32)
        if half == 0:
            slab[:, :, :] = x[b, :, 0:25, :]
        else:
            slab[:, 0:24, :] = x[b, :, 24:48, :]
        flat = slab.reshape(C, L)
        iw = np.zeros((128, IWC), np.float32)
        iw[:, 0:XOFF] = wq
        iw[0:64, XOFF : XOFF + L] = flat
        iw[64:128, XOFF : XOFF + L - 1] = flat[:, 1:]
        in_maps.append({"xw": np.ascontiguousarray(iw.astype(bf))})
    return in_maps


def _patch_edges(out, input, weight, kernels, bias):
    """Overwrite the h'=95 row and w'=95 column with edge-normalized values."""
    _, edge = _eff_weights(weight, kernels)
    x = input.astype(np.float32)
    bias32 = bias.astype(np.float32)[None, :, None]
    col47 = x[:, :, :, 47]                      # [B, C, 48]
    row47 = x[:, :, 47, :]                      # [B, C, 48]
    em = lambda M, v: np.einsum("oc,bcr->bor", M, v)
    # w'=95 column: h' even rows use Ef; h' odd rows 1..93 use Ei/Ec
    out[:, :, 0:96:2, 95] = em(edge["Ef"], col47) + bias32
    out[:, :, 1:95:2, 95] = (em(edge["Ei"], col47[:, :, 0:47])
                             + em(edge["Ec"], col47[:, :, 1:48]) + bias32)
    # h'=95 row: w' even use Rh; w' odd 1..93 use Ri/Rg
    out[:, :, 95, 0:96:2] = em(edge["Rh"], row47) + bias32
    out[:, :, 95, 1:95:2] = (em(edge["Ri"], row47[:, :, 0:47])
                             + em(edge["Rg"], row47[:, :, 1:48]) + bias32)
    # corner (95, 95)
    out[:, :, 95, 95] = (edge["Ci"] @ x[:, :, 47, 47].T).T + bias32[:, :, 0]
    return out


def kernel(input, weight, kernels, bias):
    from concourse.bass_utils import run_bass_kernel_spmd

    input = np.asarray(input)
    weight = np.asarray(weight)
    kernels = np.asarray(kernels)
    bias = np.asarray(bias)

    if "nc" not in _prog_cache:
        _prog_cache["nc"] = _build_program()
    nc = _prog_cache["nc"]

    in_maps = _make_in_maps(input, weight, kernels, bias)
    res = run_bass_kernel_spmd(nc, in_maps, core_ids=list(range(8)))

    out = np.empty((B, O, HO, WO), np.float32)
    for core in range(8):
        b, half = core // 2, core % 2
        r = np.asarray(res.results[core]["out"]).astype(np.float32)
        r = r.reshape(128, NCH, 2, 8, 48)
        rows = slice(48 * half, 48 * half + 48)
        blk = np.empty((O, 24, 2, 48, 2), np.float32)
        blk[:, :, 0, :, 0] = r[0:64, :, 0].reshape(64, 24, 48)    # ee
        blk[:, :, 0, :, 1] = r[64:128, :, 0].reshape(64, 24, 48)  # eo
        blk[:, :, 1, :, 0] = r[0:64, :, 1].reshape(64, 24, 48)    # oe
        blk[:, :, 1, :, 1] = r[64:128, :, 1].reshape(64, 24, 48)  # oo
        out[b, :, rows, :] = blk.reshape(O, 48, 96)
    out += bias.astype(np.float32)[None, :, None, None]
    _patch_edges(out, input, weight, kernels, bias)
    return out
